# revision 12
# baseline (speedup 1.0000x reference)
"""PhaseLinear Trainium2 kernel: out[b,:] = sum_p alpha(phase_b)[p] * (x[b] @ W_p.T + bias_p).

Sharding: 8 cores = 4 batch groups x 2 out_features halves.
Per core: B_s=2048, IN=1024, OUT_s=512, 4 experts.

Device algorithm (per core):
  1. alpha (B_s, 4) from phase via Catmull-Rom cubic + quadrant permutation:
     quadrant via is_ge chain, cubic via Horner with an inline-const table
     (all 4 coefficient cubics evaluated in one [128, 16, 8] pass), quadrant
     select via predicated copies over the double-width (periodic) tile.
  2. alphaT via one PE transpose -> per-chunk [4, 128] lhsT tiles.
  3. bias_blend[b,n] = alpha @ biases via tiny K=4 matmuls pipelined one
     chunk ahead of the main loop.
  4. y_p accumulated in PSUM (bf16 matmuls, K=1024, p-outer so each expert's
     PSUM finishes early; 4 banks double buffered = all 8 banks).
  5. blend on DVE: out = (((y0*a0 + bias_bl) + y1*a1) + y2*a2) + y3*a3 using
     scalar_tensor_tensor with per-partition alpha scalars.

Host-side prep (sharding/layout/dtype): inputs are pre-transposed,
pre-tiled to the exact SBUF layouts (so every DMA is a flat contiguous 2D
pattern) and activation/weights pre-cast to bf16 (the compute dtype).
"""

import numpy as np
import ml_dtypes

B, IN, OUT, NUM_CP = 8192, 1024, 1024, 4
NB, NO = 4, 2  # batch groups x out halves
BS = B // NB  # 2048 per-core batch
OS = OUT // NO  # 512 per-core out features
NCHUNK = BS // 128  # 16 b-chunks per core
KC = IN // 128  # 8 contraction chunks
NG = BS // 512  # 4 b-groups (DMA granularity)

# control_point_indices[q][j]: coeff j lands on control point CP_IDX[q, j];
# equivalently alpha[b, c] = coeff_{(c - q + 1) % 4}(t_b).
CP_IDX = [[3, 0, 1, 2], [0, 1, 2, 3], [1, 2, 3, 0], [2, 3, 0, 1]]

LAST_EXEC_NS = None
LAST_TRACE = None

_CACHE = {}


def _build(basis: np.ndarray):
    import concourse.mybir as mybir
    from concourse import bacc
    from concourse.tile import TileContext
    from concourse.masks import make_identity

    f32 = mybir.dt.float32
    bf16 = mybir.dt.bfloat16
    u8 = mybir.dt.uint8
    MULT = mybir.AluOpType.mult
    ADD = mybir.AluOpType.add
    IS_GE = mybir.AluOpType.is_ge
    IS_EQ = mybir.AluOpType.is_equal
    SUB = mybir.AluOpType.subtract

    nc = bacc.Bacc("TRN2", target_bir_lowering=False, debug=False)

    # host pre-tiled layouts: xh[m, g, k, b], wh[m, p, k, n]
    xt = nc.dram_tensor("xt", [128, NG, KC, 512], bf16, kind="ExternalInput")
    wt = nc.dram_tensor("wt", [128, NUM_CP, KC, OS], bf16, kind="ExternalInput")
    bia = nc.dram_tensor("bia", [NUM_CP, OS], f32, kind="ExternalInput")
    ph = nc.dram_tensor("ph", [128, NCHUNK], f32, kind="ExternalInput")
    out = nc.dram_tensor("out", [BS, OS], f32, kind="ExternalOutput")

    half_pi = float(np.float32(np.pi / 2.0))
    inv_half_pi = float(np.float32(1.0) / np.float32(np.pi / 2.0))
    bs = np.asarray(basis, dtype=np.float32)
    # Horner-level constant rows, periodic over j (width 8), replicated to
    # all 128 partitions: kdata[p, lvl, jj] = basis[lvl, jj % 4]
    kdata = np.tile(np.tile(bs, (1, 2))[None, :, :], (128, 1, 1)).astype(np.float32)
    kconst_dram = nc.inline_tensor(kdata, name="kconst")

    with TileContext(nc) as tc:
        with (
            tc.tile_pool(name="res", bufs=1) as res,
            tc.tile_pool(name="acc", bufs=2) as accp,
            tc.tile_pool(name="outp", bufs=3) as outp,
            tc.tile_pool(name="psum", bufs=2, space="PSUM") as psp,
        ):
            # ---- resident tiles
            w_sb = res.tile([128, NUM_CP, KC, OS], bf16, tag="w_sb", name="w_sb")
            x_sb = res.tile([128, NG, KC, 512], bf16, tag="x_sb", name="x_sb")
            bias_bl = res.tile([128, NCHUNK, OS], f32, tag="bias_bl", name="bias_bl")
            A = res.tile([128, NCHUNK, NUM_CP], f32, tag="A", name="A")
            at4 = res.tile([4, NCHUNK, 128], bf16, tag="at4", name="at4")
            at_sb = res.tile([4 * NCHUNK, 128], bf16, tag="at_sb", name="at_sb")
            biases_sb = res.tile([NUM_CP, OS], f32, tag="biases_sb", name="biases_sb")
            biases_bf = res.tile([NUM_CP, OS], bf16, tag="biases_bf", name="biases_bf")
            ph_sb = res.tile([128, NCHUNK], f32, tag="ph_sb", name="ph_sb")
            ident = res.tile([128, 128], f32, tag="ident", name="ident")
            kconst = res.tile([128, 4, 8], f32, tag="kconst", name="kconst")

            # ---- tiny DMAs first (they gate the alpha chain)
            nc.sync.dma_start(ph_sb[:], ph[:])
            nc.sync.dma_start(biases_sb[:], bia[:])
            nc.vector.tensor_copy(biases_bf[:], biases_sb[:])
            nc.sync.dma_start(kconst[:], kconst_dram[:])
            make_identity(nc, ident[:])

            # ---- input streams, contiguous 2D DMAs, issue-ordered to match
            # the p-outer consumption of chunk 0: w(p0) -> x(g0) -> w(p1..3)
            def w_dma(p, kh):
                nc.sync.dma_start(
                    w_sb[:, p, kh * 4 : (kh + 1) * 4, :],
                    wt[:, p, kh * 4 : (kh + 1) * 4, :],
                )

            def x_dma(g, kh):
                nc.sync.dma_start(
                    x_sb[:, g, kh * 4 : (kh + 1) * 4, :],
                    xt[:, g, kh * 4 : (kh + 1) * 4, :],
                )

            w_dma(0, 0)
            x_dma(0, 0)
            w_dma(0, 1)
            x_dma(0, 1)
            for p in range(1, NUM_CP):
                w_dma(p, 0)
                w_dma(p, 1)
            for g in range(1, NG):
                x_dma(g, 0)
                x_dma(g, 1)

            # ---- alpha chain: quadrant + Horner cubics on DVE, one-hot
            # masks on gpsimd (parallel)
            def small(tag, dt=f32):
                return res.tile([128, NCHUNK], dt, tag=tag, name=tag)

            quad = small("quad")
            nc.vector.tensor_scalar(quad[:], ph_sb[:], half_pi, None, IS_GE)
            nc.vector.scalar_tensor_tensor(
                quad[:], ph_sb[:], 2.0 * half_pi, quad[:], IS_GE, ADD
            )
            nc.vector.scalar_tensor_tensor(
                quad[:], ph_sb[:], 3.0 * half_pi, quad[:], IS_GE, ADD
            )
            t = small("t")
            nc.vector.scalar_tensor_tensor(
                t[:], ph_sb[:], inv_half_pi, quad[:], MULT, SUB
            )

            # quadrant one-hots (uint8 for CopyPredicated) on gpsimd
            ohs = []
            for q in range(1, 4):
                oh = small(f"oh{q}", u8)
                nc.gpsimd.tensor_scalar(oh[:], quad[:], float(q), None, IS_EQ)
                ohs.append(oh)

            # C2[:, bo, jj] = coeff_{jj % 4}(t) via Horner with broadcast consts
            C2 = res.tile([128, NCHUNK, 8], f32, tag="C2", name="C2")
            tb = t[:, :, None].to_broadcast([128, NCHUNK, 8])

            def kb(lvl):
                return kconst[:, lvl, None, :].to_broadcast([128, NCHUNK, 8])

            nc.vector.tensor_tensor(C2[:], kb(0), tb, MULT)
            nc.vector.tensor_tensor(C2[:], C2[:], kb(1), ADD)
            nc.vector.tensor_tensor(C2[:], C2[:], tb, MULT)
            nc.vector.tensor_tensor(C2[:], C2[:], kb(2), ADD)
            nc.vector.tensor_tensor(C2[:], C2[:], tb, MULT)
            nc.vector.tensor_tensor(C2[:], C2[:], kb(3), ADD)

            # A[:, bo, c] = coeff_{(c - q + 1) % 4} : shifted windows of C2
            shift = {0: 1, 1: 0, 2: 3, 3: 2}
            nc.vector.tensor_copy(A[:], C2[:, :, shift[0] : shift[0] + 4])
            for q in range(1, 4):
                nc.vector.copy_predicated(
                    A[:],
                    ohs[q - 1][:, :, None].to_broadcast([128, NCHUNK, 4]),
                    C2[:, :, shift[q] : shift[q] + 4],
                )

            # ---- bias matmul helper (K=4), pipelined one chunk ahead
            def bias_mm(bo):
                bps = psp.tile([128, OS], f32, tag=f"y{bo % 4}", name="bps")
                nc.tensor.matmul(
                    bps[:], at4[:, bo, :], biases_bf[:], start=True, stop=True
                )
                nc.scalar.copy(bias_bl[:, bo, :], bps[:])

            # ---- main loop, p-outer (each expert's PSUM closes early so the
            # DVE blend chain pipelines under the remaining matmuls)
            for bo in range(NCHUNK):
                g, sub = bo // 4, bo % 4
                psums = [
                    psp.tile([128, OS], f32, tag=f"y{p}", name=f"y{p}")
                    for p in range(4)
                ]
                for p in range(4):
                    for k in range(KC):
                        nc.tensor.matmul(
                            psums[p][:],
                            x_sb[:, g, k, sub * 128 : (sub + 1) * 128],
                            w_sb[:, p, k, :],
                            start=(k == 0),
                            stop=(k == KC - 1),
                        )

                if bo == 0:
                    # alphaT: A [128, 64] -> psum [64, 128] -> at_sb -> at4
                    at_ps = psp.tile([4 * NCHUNK, 128], f32, tag="y3", name="at_ps")
                    nc.tensor.transpose(
                        at_ps[:], A[:].rearrange("p a b -> p (a b)"), ident[:]
                    )
                    nc.vector.tensor_copy(at_sb[:], at_ps[:])
                    for b2 in range(NCHUNK):
                        nc.scalar.dma_start(
                            at4[:, b2, :], at_sb[b2 * 4 : (b2 + 1) * 4, :]
                        )
                    bias_mm(0)
                if bo < NCHUNK - 1:
                    bias_mm(bo + 1)

                a = [A[:, bo, p : p + 1] for p in range(4)]
                acc0 = accp.tile([128, OS], f32, tag="acc0", name="acc0")
                nc.vector.tensor_scalar_mul(acc0[:], psums[0][:], a[0])
                acc1 = accp.tile([128, OS], f32, tag="acc1", name="acc1")
                nc.vector.scalar_tensor_tensor(
                    acc1[:], psums[1][:], a[1], acc0[:], MULT, ADD
                )
                acc2 = accp.tile([128, OS], f32, tag="acc2", name="acc2")
                nc.vector.scalar_tensor_tensor(
                    acc2[:], psums[2][:], a[2], acc1[:], MULT, ADD
                )
                acc3 = accp.tile([128, OS], f32, tag="acc3", name="acc3")
                nc.vector.scalar_tensor_tensor(
                    acc3[:], psums[3][:], a[3], acc2[:], MULT, ADD
                )
                ot = outp.tile([128, OS], f32, tag="outsb", name="outsb")
                nc.vector.tensor_tensor(ot[:], acc3[:], bias_bl[:, bo, :], ADD)
                nc.sync.dma_start(out[bo * 128 : (bo + 1) * 128, :], ot[:])

    nc.compile()
    return nc


def _setup_trace_support():
    """Best-effort NTFF tracing under axon: register the profile hook that the
    image's antenv lacks, and neuter the artifact upload (no bucket here)."""
    try:
        import antenv.axon_hooks  # noqa: F401
    except ImportError:
        try:
            import sys
            import types

            import antenv
            from trn_agent_boot.trn_boot import _ntff_profile_via_ctypes

            hook = _ntff_profile_via_ctypes("/opt/axon/libaxon_pjrt.so")
            if hook is None:
                return False
            mod = types.ModuleType("antenv.axon_hooks")
            mod._hook = hook
            mod.get_axon_ntff_profile_hook = lambda: mod._hook
            mod.set_axon_ntff_profile_hook = lambda h: setattr(mod, "_hook", h)
            sys.modules["antenv.axon_hooks"] = mod
            antenv.axon_hooks = mod
        except Exception as e:
            print(f"trace hook setup failed: {e!r}")
            return False
    try:
        import concourse.bass_utils as bu

        bu.upload_artifacts = lambda tmpdir: str(tmpdir)
    except Exception:
        pass
    return True


def _prep_core_inputs(input, phase, weights, biases):
    """Shard + lay out host arrays to match the kernel's DMA-friendly views."""
    xt_full = input.T.astype(ml_dtypes.bfloat16)  # (IN, B)
    w_bf16 = weights.astype(ml_dtypes.bfloat16)  # (4, OUT, IN)

    in_maps = []
    for c in range(8):
        bg, oh = c // NO, c % NO
        xs = xt_full[:, bg * BS : (bg + 1) * BS]  # (IN, BS)
        # xh[m, g, k, b] = xs[k*128+m, g*512+b]
        xh = np.ascontiguousarray(
            xs.reshape(KC, 128, NG, 512).transpose(1, 2, 0, 3)
        )
        ws = w_bf16[:, oh * OS : (oh + 1) * OS, :]  # (4, OS, IN)
        # wh[m, p, k, n] = ws[p, n, k*128+m]
        wh = np.ascontiguousarray(
            ws.reshape(NUM_CP, OS, KC, 128).transpose(3, 0, 2, 1)
        )
        in_maps.append(
            {
                "xt": xh,
                "wt": wh,
                "bia": np.ascontiguousarray(biases[:, oh * OS : (oh + 1) * OS]),
                "ph": np.ascontiguousarray(
                    phase[bg * BS : (bg + 1) * BS].reshape(NCHUNK, 128).T
                ),
            }
        )
    return in_maps


def kernel(input, phase, weights, biases, basis):
    global LAST_EXEC_NS, LAST_TRACE
    from concourse.bass_utils import run_bass_kernel_spmd

    input = np.asarray(input, dtype=np.float32)
    phase = np.ascontiguousarray(np.asarray(phase, dtype=np.float32))
    weights = np.asarray(weights, dtype=np.float32)
    biases = np.asarray(biases, dtype=np.float32)
    basis = np.asarray(basis, dtype=np.float32)

    key = basis.tobytes()
    if key not in _CACHE:
        _CACHE[key] = _build(basis)
    nc = _CACHE[key]

    in_maps = _prep_core_inputs(input, phase, weights, biases)

    res = run_bass_kernel_spmd(nc, in_maps, core_ids=list(range(8)), trace=False)
    LAST_EXEC_NS = res.exec_time_ns
    LAST_TRACE = res.instructions_and_trace[1] if res.instructions_and_trace else None

    full = np.empty((B, OUT), dtype=np.float32)
    for c in range(8):
        bg, oh = c // NO, c % NO
        full[bg * BS : (bg + 1) * BS, oh * OS : (oh + 1) * OS] = res.results[c]["out"]
    return full


# revision 14
# speedup vs baseline: 1.1908x; 1.1908x over previous
"""PhaseLinear Trainium2 kernel: out[b,:] = sum_p alpha(phase_b)[p] * (x[b] @ W_p.T + bias_p).

Sharding: 8 cores = 4 batch groups x 2 out_features halves.
Per core: B_s=2048, IN=1024, OUT_s=512, 4 experts.

Device algorithm (per core):
  1. alpha (B_s, 4) from phase via Catmull-Rom cubic + quadrant permutation:
     quadrant via is_ge chain, cubic via Horner with an inline-const table
     (all 4 coefficient cubics evaluated in one [128, 16, 8] pass), quadrant
     select via predicated copies over the double-width (periodic) tile.
  2. alphaT via one PE transpose -> per-chunk [4, 128] lhsT tiles.
  3. bias_blend[b,n] = alpha @ biases via tiny K=4 matmuls pipelined one
     chunk ahead of the main loop.
  4. y_p accumulated in PSUM (bf16 matmuls, K=1024, p-outer so each expert's
     PSUM finishes early; 4 banks double buffered = all 8 banks).
  5. blend on DVE: out = (((y0*a0 + bias_bl) + y1*a1) + y2*a2) + y3*a3 using
     scalar_tensor_tensor with per-partition alpha scalars.

Host-side prep (sharding/layout/dtype): inputs are pre-transposed,
pre-tiled to the exact SBUF layouts (so every DMA is a flat contiguous 2D
pattern) and activation/weights pre-cast to bf16 (the compute dtype).
"""

import numpy as np
import ml_dtypes

B, IN, OUT, NUM_CP = 8192, 1024, 1024, 4
NB, NO = 4, 2  # batch groups x out halves
BS = B // NB  # 2048 per-core batch
OS = OUT // NO  # 512 per-core out features
NCHUNK = BS // 128  # 16 b-chunks per core
KC = IN // 128  # 8 contraction chunks
NG = BS // 512  # 4 b-groups (DMA granularity)

# control_point_indices[q][j]: coeff j lands on control point CP_IDX[q, j];
# equivalently alpha[b, c] = coeff_{(c - q + 1) % 4}(t_b).
CP_IDX = [[3, 0, 1, 2], [0, 1, 2, 3], [1, 2, 3, 0], [2, 3, 0, 1]]

LAST_EXEC_NS = None
LAST_TRACE = None

_CACHE = {}


def _build(basis: np.ndarray):
    import concourse.mybir as mybir
    from concourse import bacc
    from concourse.tile import TileContext
    from concourse.masks import make_identity

    f32 = mybir.dt.float32
    bf16 = mybir.dt.bfloat16
    u8 = mybir.dt.uint8
    MULT = mybir.AluOpType.mult
    ADD = mybir.AluOpType.add
    IS_GE = mybir.AluOpType.is_ge
    IS_EQ = mybir.AluOpType.is_equal
    SUB = mybir.AluOpType.subtract

    nc = bacc.Bacc("TRN2", target_bir_lowering=False, debug=False)

    # host pre-tiled layouts: xh[m, g, k, b], wh[m, p, k, n]
    xt = nc.dram_tensor("xt", [128, NG, KC, 512], bf16, kind="ExternalInput")
    wt = nc.dram_tensor("wt", [128, NUM_CP, KC, OS], bf16, kind="ExternalInput")
    bia = nc.dram_tensor("bia", [NUM_CP, OS], f32, kind="ExternalInput")
    ph = nc.dram_tensor("ph", [128, NCHUNK], f32, kind="ExternalInput")
    out = nc.dram_tensor("out", [BS, OS], f32, kind="ExternalOutput")

    half_pi = float(np.float32(np.pi / 2.0))
    inv_half_pi = float(np.float32(1.0) / np.float32(np.pi / 2.0))
    bs = np.asarray(basis, dtype=np.float32)
    # Horner-level constant rows, periodic over j (width 8), replicated to
    # all 128 partitions: kdata[p, lvl, jj] = basis[lvl, jj % 4]
    kdata = np.tile(np.tile(bs, (1, 2))[None, :, :], (128, 1, 1)).astype(np.float32)
    kconst_dram = nc.inline_tensor(kdata, name="kconst")

    with TileContext(nc) as tc:
        with (
            tc.tile_pool(name="res", bufs=1) as res,
            tc.tile_pool(name="acc", bufs=2) as accp,
            tc.tile_pool(name="outp", bufs=3) as outp,
            tc.tile_pool(name="psum", bufs=2, space="PSUM") as psp,
        ):
            # ---- resident tiles
            w_sb = res.tile([128, NUM_CP, KC, OS], bf16, tag="w_sb", name="w_sb")
            x_sb = res.tile([128, NG, KC, 512], bf16, tag="x_sb", name="x_sb")
            bias_bl = res.tile([128, NCHUNK, OS], f32, tag="bias_bl", name="bias_bl")
            A = res.tile([128, NCHUNK, NUM_CP], f32, tag="A", name="A")
            at4 = res.tile([4, NCHUNK, 128], bf16, tag="at4", name="at4")
            at_sb = res.tile([4 * NCHUNK, 128], bf16, tag="at_sb", name="at_sb")
            biases_sb = res.tile([NUM_CP, OS], f32, tag="biases_sb", name="biases_sb")
            biases_bf = res.tile([NUM_CP, OS], bf16, tag="biases_bf", name="biases_bf")
            ph_sb = res.tile([128, NCHUNK], f32, tag="ph_sb", name="ph_sb")
            ident = res.tile([128, 128], f32, tag="ident", name="ident")
            kconst = res.tile([128, 4, 8], f32, tag="kconst", name="kconst")

            # ---- tiny DMAs first (they gate the alpha chain)
            nc.sync.dma_start(ph_sb[:], ph[:])
            nc.sync.dma_start(biases_sb[:], bia[:])
            nc.vector.tensor_copy(biases_bf[:], biases_sb[:])
            nc.sync.dma_start(kconst[:], kconst_dram[:])
            make_identity(nc, ident[:])

            # ---- input streams, contiguous 2D DMAs, issue-ordered to match
            # the p-outer consumption of chunk 0: w(p0) -> x(g0) -> w(p1..3)
            def w_dma(p, kh):
                nc.sync.dma_start(
                    w_sb[:, p, kh * 4 : (kh + 1) * 4, :],
                    wt[:, p, kh * 4 : (kh + 1) * 4, :],
                )

            def x_dma(g, kh):
                nc.sync.dma_start(
                    x_sb[:, g, kh * 4 : (kh + 1) * 4, :],
                    xt[:, g, kh * 4 : (kh + 1) * 4, :],
                )

            w_dma(0, 0)
            x_dma(0, 0)
            w_dma(0, 1)
            x_dma(0, 1)
            for p in range(1, NUM_CP):
                w_dma(p, 0)
                w_dma(p, 1)
            for g in range(1, NG):
                x_dma(g, 0)
                x_dma(g, 1)

            # ---- alpha chain: quadrant + Horner cubics on DVE, one-hot
            # masks on gpsimd (parallel)
            def small(tag, dt=f32):
                return res.tile([128, NCHUNK], dt, tag=tag, name=tag)

            quad = small("quad")
            nc.vector.tensor_scalar(quad[:], ph_sb[:], half_pi, None, IS_GE)
            nc.vector.scalar_tensor_tensor(
                quad[:], ph_sb[:], 2.0 * half_pi, quad[:], IS_GE, ADD
            )
            nc.vector.scalar_tensor_tensor(
                quad[:], ph_sb[:], 3.0 * half_pi, quad[:], IS_GE, ADD
            )
            t = small("t")
            nc.vector.scalar_tensor_tensor(
                t[:], ph_sb[:], inv_half_pi, quad[:], MULT, SUB
            )

            # quadrant one-hots (uint8 for CopyPredicated) on gpsimd
            ohs = []
            for q in range(1, 4):
                oh = small(f"oh{q}", u8)
                nc.gpsimd.tensor_scalar(oh[:], quad[:], float(q), None, IS_EQ)
                ohs.append(oh)

            # C2[:, bo, jj] = coeff_{jj % 4}(t) via Horner with broadcast consts
            C2 = res.tile([128, NCHUNK, 8], f32, tag="C2", name="C2")
            tb = t[:, :, None].to_broadcast([128, NCHUNK, 8])

            def kb(lvl):
                return kconst[:, lvl, None, :].to_broadcast([128, NCHUNK, 8])

            nc.vector.tensor_tensor(C2[:], kb(0), tb, MULT)
            nc.vector.tensor_tensor(C2[:], C2[:], kb(1), ADD)
            nc.vector.tensor_tensor(C2[:], C2[:], tb, MULT)
            nc.vector.tensor_tensor(C2[:], C2[:], kb(2), ADD)
            nc.vector.tensor_tensor(C2[:], C2[:], tb, MULT)
            nc.vector.tensor_tensor(C2[:], C2[:], kb(3), ADD)

            # A[:, bo, c] = coeff_{(c - q + 1) % 4} : shifted windows of C2
            shift = {0: 1, 1: 0, 2: 3, 3: 2}
            nc.vector.tensor_copy(A[:], C2[:, :, shift[0] : shift[0] + 4])
            for q in range(1, 4):
                nc.vector.copy_predicated(
                    A[:],
                    ohs[q - 1][:, :, None].to_broadcast([128, NCHUNK, 4]),
                    C2[:, :, shift[q] : shift[q] + 4],
                )

            # ---- bias matmul helper (K=4), pipelined one chunk ahead
            def bias_mm(bo):
                bps = psp.tile([128, OS], f32, tag=f"y{bo % 4}", name="bps")
                nc.tensor.matmul(
                    bps[:], at4[:, bo, :], biases_bf[:], start=True, stop=True
                )
                nc.scalar.copy(bias_bl[:, bo, :], bps[:])

            # ---- main loop, p-outer (each expert's PSUM closes early so the
            # DVE blend chain pipelines under the remaining matmuls)
            for bo in range(NCHUNK):
                g, sub = bo // 4, bo % 4
                psums = [
                    psp.tile([128, OS], f32, tag=f"y{p}", name=f"y{p}")
                    for p in range(4)
                ]
                for p in range(4):
                    for k in range(KC):
                        nc.tensor.matmul(
                            psums[p][:],
                            x_sb[:, g, k, sub * 128 : (sub + 1) * 128],
                            w_sb[:, p, k, :],
                            start=(k == 0),
                            stop=(k == KC - 1),
                        )

                if bo == 0:
                    # alphaT: A [128, 64] -> psum [64, 128] -> at_sb -> at4
                    at_ps = psp.tile([4 * NCHUNK, 128], f32, tag="y3", name="at_ps")
                    nc.tensor.transpose(
                        at_ps[:], A[:].rearrange("p a b -> p (a b)"), ident[:]
                    )
                    nc.vector.tensor_copy(at_sb[:], at_ps[:])
                    for b2 in range(NCHUNK):
                        nc.scalar.dma_start(
                            at4[:, b2, :], at_sb[b2 * 4 : (b2 + 1) * 4, :]
                        )
                    bias_mm(0)
                if bo < NCHUNK - 1:
                    bias_mm(bo + 1)

                a = [A[:, bo, p : p + 1] for p in range(4)]
                acc0 = accp.tile([128, OS], f32, tag="acc0", name="acc0")
                if bo < 4:
                    # pipeline fill: don't gate the first PSUM release on the
                    # alphaT/bias path (bias added as a 5th op below)
                    nc.vector.tensor_scalar_mul(acc0[:], psums[0][:], a[0])
                else:
                    nc.vector.scalar_tensor_tensor(
                        acc0[:], psums[0][:], a[0], bias_bl[:, bo, :], MULT, ADD
                    )
                acc1 = accp.tile([128, OS], f32, tag="acc1", name="acc1")
                nc.vector.scalar_tensor_tensor(
                    acc1[:], psums[1][:], a[1], acc0[:], MULT, ADD
                )
                acc2 = accp.tile([128, OS], f32, tag="acc2", name="acc2")
                nc.vector.scalar_tensor_tensor(
                    acc2[:], psums[2][:], a[2], acc1[:], MULT, ADD
                )
                if bo < 4:
                    acc3 = accp.tile([128, OS], f32, tag="acc3", name="acc3")
                    nc.vector.scalar_tensor_tensor(
                        acc3[:], psums[3][:], a[3], acc2[:], MULT, ADD
                    )
                    ot = outp.tile([128, OS], f32, tag="outsb", name="outsb")
                    nc.vector.tensor_tensor(ot[:], acc3[:], bias_bl[:, bo, :], ADD)
                else:
                    ot = outp.tile([128, OS], f32, tag="outsb", name="outsb")
                    nc.vector.scalar_tensor_tensor(
                        ot[:], psums[3][:], a[3], acc2[:], MULT, ADD
                    )
                nc.sync.dma_start(out[bo * 128 : (bo + 1) * 128, :], ot[:])

    nc.compile()
    return nc


def _setup_trace_support():
    """Best-effort NTFF tracing under axon: register the profile hook that the
    image's antenv lacks, and neuter the artifact upload (no bucket here)."""
    try:
        import antenv.axon_hooks  # noqa: F401
    except ImportError:
        try:
            import sys
            import types

            import antenv
            from trn_agent_boot.trn_boot import _ntff_profile_via_ctypes

            hook = _ntff_profile_via_ctypes("/opt/axon/libaxon_pjrt.so")
            if hook is None:
                return False
            mod = types.ModuleType("antenv.axon_hooks")
            mod._hook = hook
            mod.get_axon_ntff_profile_hook = lambda: mod._hook
            mod.set_axon_ntff_profile_hook = lambda h: setattr(mod, "_hook", h)
            sys.modules["antenv.axon_hooks"] = mod
            antenv.axon_hooks = mod
        except Exception as e:
            print(f"trace hook setup failed: {e!r}")
            return False
    try:
        import concourse.bass_utils as bu

        bu.upload_artifacts = lambda tmpdir: str(tmpdir)
    except Exception:
        pass
    return True


def _prep_core_inputs(input, phase, weights, biases):
    """Shard + lay out host arrays to match the kernel's DMA-friendly views."""
    xt_full = input.T.astype(ml_dtypes.bfloat16)  # (IN, B)
    w_bf16 = weights.astype(ml_dtypes.bfloat16)  # (4, OUT, IN)

    in_maps = []
    for c in range(8):
        bg, oh = c // NO, c % NO
        xs = xt_full[:, bg * BS : (bg + 1) * BS]  # (IN, BS)
        # xh[m, g, k, b] = xs[k*128+m, g*512+b]
        xh = np.ascontiguousarray(
            xs.reshape(KC, 128, NG, 512).transpose(1, 2, 0, 3)
        )
        ws = w_bf16[:, oh * OS : (oh + 1) * OS, :]  # (4, OS, IN)
        # wh[m, p, k, n] = ws[p, n, k*128+m]
        wh = np.ascontiguousarray(
            ws.reshape(NUM_CP, OS, KC, 128).transpose(3, 0, 2, 1)
        )
        in_maps.append(
            {
                "xt": xh,
                "wt": wh,
                "bia": np.ascontiguousarray(biases[:, oh * OS : (oh + 1) * OS]),
                "ph": np.ascontiguousarray(
                    phase[bg * BS : (bg + 1) * BS].reshape(NCHUNK, 128).T
                ),
            }
        )
    return in_maps


def kernel(input, phase, weights, biases, basis):
    global LAST_EXEC_NS, LAST_TRACE
    from concourse.bass_utils import run_bass_kernel_spmd

    input = np.asarray(input, dtype=np.float32)
    phase = np.ascontiguousarray(np.asarray(phase, dtype=np.float32))
    weights = np.asarray(weights, dtype=np.float32)
    biases = np.asarray(biases, dtype=np.float32)
    basis = np.asarray(basis, dtype=np.float32)

    key = basis.tobytes()
    if key not in _CACHE:
        _CACHE[key] = _build(basis)
    nc = _CACHE[key]

    in_maps = _prep_core_inputs(input, phase, weights, biases)

    res = run_bass_kernel_spmd(nc, in_maps, core_ids=list(range(8)), trace=False)
    LAST_EXEC_NS = res.exec_time_ns
    LAST_TRACE = res.instructions_and_trace[1] if res.instructions_and_trace else None

    full = np.empty((B, OUT), dtype=np.float32)
    for c in range(8):
        bg, oh = c // NO, c % NO
        full[bg * BS : (bg + 1) * BS, oh * OS : (oh + 1) * OS] = res.results[c]["out"]
    return full
